# revision 6
# baseline (speedup 1.0000x reference)
"""Self-contained GraphSAGE (3-layer, mean-aggr) Bass/Tile kernel for 8x TRN2.

kernel(**inputs) takes the FULL inputs (x [50000,128] f32, edge_index
[2,800000] i32, weights/biases) and returns the full [50000,64] f32 output.

Sharding: nodes split 8 ways; edges partitioned by destination shard; per
layer an AllGather of bf16 features; per-window bulk dma_gather of source
rows (int16 indices, split at row 32768 into lo/hi range gathers) and a
one-hot-matmul segment-mean on the tensor engine.
"""


import math

import numpy as np
from ml_dtypes import bfloat16

import concourse.bacc as bacc
import concourse.bass as bass
import concourse.tile as tile
from concourse import library_config, mybir
from concourse.bass_utils import run_bass_kernel_spmd


def _ensure_ntff_hook():
    """The agent image's ``antenv`` lacks ``axon_hooks``; synthesize it and
    install the ctypes-based NTFF profile hook so trace=True works."""
    try:
        from antenv.axon_hooks import get_axon_ntff_profile_hook  # noqa: F401
        return
    except ImportError:
        pass
    import sys
    import types

    mod = types.ModuleType("antenv.axon_hooks")
    _hook = [None]
    mod.set_axon_ntff_profile_hook = lambda h: _hook.__setitem__(0, h)
    mod.get_axon_ntff_profile_hook = lambda: _hook[0]
    sys.modules["antenv.axon_hooks"] = mod
    try:
        import antenv

        antenv.axon_hooks = mod
    except ImportError:
        pass
    try:
        from trn_agent_boot.trn_boot import _ntff_profile_via_ctypes

        so_path = "/opt/axon/libaxon_pjrt.so"
        hook = _ntff_profile_via_ctypes(so_path)
        if hook is not None:
            mod.set_axon_ntff_profile_hook(hook)
    except Exception:
        pass


_ensure_ntff_hook()

F32 = mybir.dt.float32
BF16 = mybir.dt.bfloat16
I16 = mybir.dt.int16
AF = mybir.ActivationFunctionType
OP = mybir.AluOpType

WN = 128   # window (dst-node tile) size
D = 128    # feature dim (layers 0/1 output, all layer inputs)
DOUT = 64
SPLIT = 32768  # int16 index range split
GB = 7     # windows per gather batch


def host_prep(x, edge_index, n_cores):
    """Build per-core host-side arrays. Edges are bucketed per dst window,
    split into lo/hi src ranges (int16 index limit), sorted by src for HBM
    locality, and laid out in gather-batch order."""
    N, d = x.shape
    assert d == D
    NS = N // n_cores
    W = math.ceil(NS / WN)
    src = edge_index[0].astype(np.int64)
    dst = edge_index[1].astype(np.int64)

    deg = np.bincount(dst, minlength=N).astype(np.float32)
    inv = (1.0 / np.maximum(deg, 1.0)).astype(np.float32)

    order = np.argsort(dst, kind="stable")
    srcs = src[order]
    dsts = dst[order]
    bounds = np.searchsorted(dsts, np.arange(n_cores + 1) * NS)

    # per (core, window, lo/hi) edge lists
    per_core = []
    nlo = np.zeros((n_cores, W), dtype=np.int64)
    nhi = np.zeros((n_cores, W), dtype=np.int64)
    for k in range(n_cores):
        lo_, hi_ = bounds[k], bounds[k + 1]
        es = srcs[lo_:hi_]
        ed = dsts[lo_:hi_] - k * NS
        win = ed // WN
        wstart = np.searchsorted(win, np.arange(W + 1))
        wins = []
        for w in range(W):
            a, b = wstart[w], wstart[w + 1]
            ws, wd = es[a:b], ed[a:b] % WN
            is_lo = ws < SPLIT
            lo_s, lo_d = ws[is_lo], wd[is_lo]
            hi_s, hi_d = ws[~is_lo], wd[~is_lo]
            olo = np.argsort(lo_s, kind="stable")
            ohi = np.argsort(hi_s, kind="stable")
            wins.append((lo_s[olo], lo_d[olo], hi_s[ohi], hi_d[ohi]))
            nlo[k, w] = lo_s.shape[0]
            nhi[k, w] = hi_s.shape[0]
        per_core.append(wins)

    cpwl = np.maximum(1, np.ceil(nlo.max(axis=0) / 128).astype(np.int64))
    cpwh = np.ceil(nhi.max(axis=0) / 128).astype(np.int64)

    # global chunk-column layout, per batch: [lo chunks per window][hi chunks]
    batches = []  # (w0, w1, lo_start, lo_n, hi_start, hi_n)
    lo_col = np.zeros(W, dtype=np.int64)
    hi_col = np.zeros(W, dtype=np.int64)
    cur = 0
    for b0 in range(0, W, GB):
        b1 = min(b0 + GB, W)
        lo_start = cur
        for w in range(b0, b1):
            lo_col[w] = cur
            cur += cpwl[w]
        hi_start = cur
        for w in range(b0, b1):
            hi_col[w] = cur
            cur += cpwh[w]
        batches.append(
            (b0, b1, int(lo_start), int(hi_start - lo_start), int(hi_start),
             int(cur - hi_start))
        )
    T = int(cur)

    idx_l, dstl_l, invd_l, xt_l = [], [], [], []
    for k in range(n_cores):
        idx_flat = np.zeros(T * 128, dtype=np.int16)
        dst_flat = np.full(T * 128, 240.0, dtype=np.float32)
        for w in range(W):
            lo_s, lo_d, hi_s, hi_d = per_core[k][w]
            s0 = lo_col[w] * 128
            idx_flat[s0 : s0 + len(lo_s)] = lo_s.astype(np.int16)
            dst_flat[s0 : s0 + len(lo_d)] = lo_d
            s0 = hi_col[w] * 128
            idx_flat[s0 : s0 + len(hi_s)] = (hi_s - SPLIT).astype(np.int16)
            dst_flat[s0 : s0 + len(hi_d)] = hi_d
        idx_l.append(
            np.ascontiguousarray(np.tile(idx_flat.reshape(T * 8, 16).T, (8, 1)))
        )
        dstl_l.append(
            np.ascontiguousarray(dst_flat.reshape(T, 128).T.astype(bfloat16))
        )

        v = np.zeros(W * WN, dtype=np.float32)
        v[:NS] = inv[k * NS : (k + 1) * NS]
        invd_l.append(np.ascontiguousarray(np.broadcast_to(v, (128, W * WN))))

        xt = np.zeros((128, W * WN), dtype=np.float32)
        xt[:, :NS] = x[k * NS : (k + 1) * NS].T
        xt_l.append(xt)

    iota = np.ascontiguousarray(
        np.broadcast_to(np.arange(WN, dtype=np.float32), (128, WN)).astype(bfloat16)
    )
    return dict(
        N=N, NS=NS, W=W, T=T, n_cores=n_cores,
        CPWL=tuple(int(c) for c in cpwl), CPWH=tuple(int(c) for c in cpwh),
        BATCHES=tuple(batches),
        LO_COL=tuple(int(c) for c in lo_col), HI_COL=tuple(int(c) for c in hi_col),
        idx=idx_l, dstl=dstl_l, invd=invd_l, xt=xt_l,
        x_bf16=np.ascontiguousarray(x.astype(bfloat16)).view(np.float32),
        iota=iota,
    )


def build_program(N, NS, W, T, CPWL, CPWH, BATCHES, LO_COL, HI_COL, n_cores,
                  mm_bufs=2, g_bufs=2, shared_ag=True):
    """Build the Bass/Tile SPMD program."""
    nc = bacc.Bacc(
        "TRN2", target_bir_lowering=False, debug=False, num_devices=n_cores,
        num_swdge_queues=4,
    )

    # ---- I/O ----
    xg = nc.dram_tensor("xg", [N, D // 2], F32, kind="ExternalInput")
    xt_in = nc.dram_tensor("xt", [128, W * WN], F32, kind="ExternalInput")
    idx_in = nc.dram_tensor("idx", [128, T * 8], I16, kind="ExternalInput")
    dstl_in = nc.dram_tensor("dstl", [128, T], BF16, kind="ExternalInput")
    invd_in = nc.dram_tensor("invd", [128, W * WN], F32, kind="ExternalInput")
    iota_in = nc.dram_tensor("iota", [128, WN], BF16, kind="ExternalInput")
    w_in = {}
    for i, do in ((0, D), (1, D), (2, DOUT)):
        w_in[f"wl{i}"] = nc.dram_tensor(f"wl{i}", [D, do], F32, kind="ExternalInput")
        w_in[f"wr{i}"] = nc.dram_tensor(f"wr{i}", [D, do], F32, kind="ExternalInput")
    bl0_in = nc.dram_tensor("bl0", [128, 1], F32, kind="ExternalInput")
    bl1_in = nc.dram_tensor("bl1", [128, 1], F32, kind="ExternalInput")
    b2b_in = nc.dram_tensor("b2b", [128, DOUT], F32, kind="ExternalInput")
    ident_in = nc.dram_tensor("ident", [128, 128], F32, kind="ExternalInput")
    out = nc.dram_tensor("out", [NS, DOUT], F32, kind="ExternalOutput")

    groups = [list(range(n_cores))]

    with tile.TileContext(nc) as tc:
        with (
            tc.tile_pool(name="const", bufs=1) as cpool,
            tc.tile_pool(name="state", bufs=1) as spool,
            tc.tile_pool(name="gather", bufs=g_bufs) as gpool,
            tc.tile_pool(name="pbuild", bufs=4) as ppool,
            tc.tile_pool(name="small", bufs=mm_bufs * 2) as smpool,
            tc.tile_pool(name="psA", bufs=mm_bufs, space="PSUM") as psA,
            tc.tile_pool(name="psY", bufs=mm_bufs, space="PSUM") as psY,
            tc.tile_pool(name="psR", bufs=mm_bufs, space="PSUM") as psR,
            tc.tile_pool(name="dram", bufs=1, space="DRAM") as dpool,
        ):
            nc.gpsimd.load_library(library_config.mlp)

            # ---- constants / resident state ----
            iota_sb = cpool.tile([128, WN], BF16)
            nc.sync.dma_start(out=iota_sb[:], in_=iota_in[:, :])
            ident_sb = cpool.tile([128, 128], F32)
            nc.sync.dma_start(out=ident_sb[:], in_=ident_in[:, :])
            idx_sb = cpool.tile([128, T * 8], I16)
            nc.sync.dma_start(out=idx_sb[:], in_=idx_in[:, :])
            dstl_sb = cpool.tile([128, T], BF16)
            nc.sync.dma_start(out=dstl_sb[:], in_=dstl_in[:, :])
            invd_sb = cpool.tile([128, W * WN], F32)
            nc.sync.dma_start(out=invd_sb[:], in_=invd_in[:, :])
            w_sb = {}
            for name, t in w_in.items():
                w_sb[name] = cpool.tile(list(t.shape), F32, name=f"{name}_sb")
                nc.sync.dma_start(out=w_sb[name][:], in_=t[:, :])
            bl_sb = [cpool.tile([128, 1], F32, name=f"blc{i}_sb") for i in range(2)]
            nc.sync.dma_start(out=bl_sb[0][:], in_=bl0_in[:, :])
            nc.sync.dma_start(out=bl_sb[1][:], in_=bl1_in[:, :])
            b2b_sb = cpool.tile([128, DOUT], F32)
            nc.sync.dma_start(out=b2b_sb[:], in_=b2b_in[:, :])

            ht = [
                spool.tile([128, W * WN], F32, name="ht0"),
                spool.tile([128, W * WN], F32, name="ht1"),
            ]
            nc.sync.dma_start(out=ht[0][:], in_=xt_in[:, :])

            ag_in = dpool.tile([NS, D // 2], F32, name="ag_in")
            ag_space = "Shared" if (n_cores > 4 and shared_ag) else "Local"
            h_full = [
                dpool.tile([N, D // 2], F32, name="h1", addr_space=ag_space),
                dpool.tile([N, D // 2], F32, name="h2", addr_space=ag_space),
            ]

            last_rows = NS - (W - 1) * WN
            self_q = [0]  # SWDGE queue rotation counter

            for L in range(3):
                table = xg if L == 0 else h_full[L - 1]
                cur = ht[L % 2]
                nxt = ht[(L + 1) % 2]
                wl = w_sb[f"wl{L}"]
                wr = w_sb[f"wr{L}"]
                tab_lo = table[:SPLIT, :].bitcast(BF16)
                tab_hi = table[SPLIT:, :].bitcast(BF16)
                for (b0, b1, lo_start, lo_n, hi_start, hi_n) in BATCHES:
                    Tb = lo_n + hi_n
                    mw = gpool.tile([128, Tb, 128], BF16, name="mw", tag="mw")

                    # slice each lo/hi run into <=8-chunk (1024-idx) calls —
                    # the ucode packet limit — rotated over the 4 SWDGE queues
                    def gcalls(gstart, nch, col0, tab_ap, mw=mw):
                        c = 0
                        while c < nch:
                            pc = min(32, nch - c)
                            g0 = gstart + c
                            nc.gpsimd.dma_gather(
                                mw[:, col0 + c : col0 + c + pc, :], tab_ap,
                                idx_sb[:, g0 * 8 : (g0 + pc) * 8],
                                pc * 128, pc * 128, 128,
                                single_packet=False,
                                queue_num=self_q[0] % 4,
                            )
                            self_q[0] += 1
                            c += pc

                    gcalls(lo_start, lo_n, 0, tab_lo)
                    gcalls(hi_start, hi_n, lo_n, tab_hi)
                    for w in range(b0, b1):
                        rows = WN if w < W - 1 else last_rows
                        cl, ch = CPWL[w], CPWH[w]
                        NCH = cl + ch
                        # one-hot P for this window's chunks (lo then hi)
                        pw = ppool.tile([128, NCH * WN], BF16, name="pw", tag="pw")
                        nc.vector.tensor_tensor(
                            out=pw[:, : cl * WN].rearrange(
                                "p (c n) -> p c n", n=WN
                            ),
                            in0=dstl_sb[
                                :, LO_COL[w] : LO_COL[w] + cl, None
                            ].to_broadcast([128, cl, WN]),
                            in1=iota_sb[:, None, :].to_broadcast([128, cl, WN]),
                            op=OP.is_equal,
                        )
                        if ch:
                            nc.vector.tensor_tensor(
                                out=pw[:, cl * WN :].rearrange(
                                    "p (c n) -> p c n", n=WN
                                ),
                                in0=dstl_sb[
                                    :, HI_COL[w] : HI_COL[w] + ch, None
                                ].to_broadcast([128, ch, WN]),
                                in1=iota_sb[:, None, :].to_broadcast([128, ch, WN]),
                                op=OP.is_equal,
                            )
                        # segment-sum: PSUM_A[feat, node] += M_c.T @ P_c
                        pa = psA.tile([128, WN], F32, name="pa")
                        mw_lo0 = LO_COL[w] - lo_start
                        mw_hi0 = lo_n + (HI_COL[w] - hi_start)
                        for c in range(NCH):
                            mcol = (mw_lo0 + c) if c < cl else (mw_hi0 + c - cl)
                            nc.tensor.matmul(
                                out=pa[:],
                                lhsT=mw[:, mcol, :],
                                rhs=pw[:, c * WN : (c + 1) * WN],
                                start=(c == 0),
                                stop=(c == NCH - 1),
                            )
                        # normalize (segment mean) while copying PSUM->SBUF
                        aggt = smpool.tile([128, WN], F32, name="aggt")
                        nc.vector.tensor_tensor(
                            out=aggt[:],
                            in0=pa[:],
                            in1=invd_sb[:, w * WN : (w + 1) * WN],
                            op=OP.mult,
                        )
                        if L < 2:
                            # yT = Wl.T @ aggT + Wr.T @ hT_win
                            py = psY.tile([128, WN], F32, name="py")
                            nc.tensor.matmul(
                                out=py[:], lhsT=wl[:], rhs=aggt[:],
                                start=True, stop=False,
                            )
                            nc.tensor.matmul(
                                out=py[:],
                                lhsT=wr[:],
                                rhs=cur[:, w * WN : (w + 1) * WN],
                                start=False,
                                stop=True,
                            )
                            # hT_next = relu(yT + b) (bias per-partition)
                            nc.scalar.activation(
                                out=nxt[:, w * WN : (w + 1) * WN],
                                in_=py[:],
                                func=AF.Relu,
                                bias=bl_sb[L][:, :1],
                            )
                            # row-major bf16 copy for the allgather input
                            pr = psR.tile([128, WN], F32, name="pr")
                            nc.tensor.transpose(
                                out=pr[:],
                                in_=nxt[:, w * WN : (w + 1) * WN],
                                identity=ident_sb[:],
                            )
                            hrow = smpool.tile([128, D], BF16, name="hrow")
                            nc.vector.tensor_copy(out=hrow[:], in_=pr[:])
                            nc.sync.dma_start(
                                out=ag_in[w * WN : w * WN + rows, :],
                                in_=hrow[:rows, :].bitcast(F32),
                            )
                        else:
                            # final layer: out = aggT.T@Wl2 + hT.T@Wr2 + b2
                            pf = psY.tile([128, DOUT], F32, name="pf")
                            nc.tensor.matmul(
                                out=pf[:], lhsT=aggt[:], rhs=w_sb["wl2"][:],
                                start=True, stop=False,
                            )
                            nc.tensor.matmul(
                                out=pf[:],
                                lhsT=cur[:, w * WN : (w + 1) * WN],
                                rhs=w_sb["wr2"][:],
                                start=False,
                                stop=True,
                            )
                            osb = smpool.tile([128, DOUT], F32, name="osb")
                            nc.vector.tensor_tensor(
                                out=osb[:], in0=pf[:], in1=b2b_sb[:], op=OP.add
                            )
                            nc.sync.dma_start(
                                out=out[w * WN : w * WN + rows, :],
                                in_=osb[:rows, :],
                            )
                if L < 2:
                    nc.gpsimd.collective_compute(
                        "AllGather",
                        OP.bypass,
                        replica_groups=groups,
                        ins=[ag_in[:, :]],
                        outs=[h_full[L][:, :]],
                    )

    nc.compile()
    return nc


def make_in_maps(prep, params):
    """params: dict with Wl0,bl0,Wr0,...  Returns list of per-core in_maps."""
    n_cores = prep["n_cores"]
    ident = np.eye(128, dtype=np.float32)
    common = dict(
        xg=prep["x_bf16"],
        iota=prep["iota"],
        ident=ident,
        bl0=np.asarray(params["bl0"], np.float32).reshape(128, 1),
        bl1=np.asarray(params["bl1"], np.float32).reshape(128, 1),
        b2b=np.ascontiguousarray(
            np.broadcast_to(np.asarray(params["bl2"], np.float32), (128, DOUT))
        ),
    )
    for i in range(3):
        common[f"wl{i}"] = np.asarray(params[f"Wl{i}"], np.float32)
        common[f"wr{i}"] = np.asarray(params[f"Wr{i}"], np.float32)
    return [
        dict(
            common,
            xt=prep["xt"][k],
            idx=prep["idx"][k],
            dstl=prep["dstl"][k],
            invd=prep["invd"][k],
        )
        for k in range(n_cores)
    ]


def _build_key(prep):
    return (prep["N"], prep["NS"], prep["W"], prep["T"], prep["CPWL"],
            prep["CPWH"], prep["BATCHES"], prep["LO_COL"], prep["HI_COL"])


def run(x, edge_index, params, n_cores=8, trace=False, prep=None, nc=None):
    if prep is None:
        prep = host_prep(np.asarray(x, np.float32), np.asarray(edge_index), n_cores)
    if nc is None:
        nc = build_program(*_build_key(prep), n_cores)
    in_maps = make_in_maps(prep, params)
    res = run_bass_kernel_spmd(
        nc, in_maps, core_ids=list(range(n_cores)), trace=trace
    )
    outs = [res.results[k]["out"] for k in range(n_cores)]
    return np.concatenate(outs, axis=0), res


_CACHE = {}

N_NODES = 50000
N_EDGES = 800000
N_CORES = 8


def kernel(**inputs):
    x = np.asarray(inputs["x"], dtype=np.float32)
    edge_index = np.asarray(inputs["edge_index"])
    params = {k: np.asarray(v) for k, v in inputs.items()
              if k not in ("x", "edge_index")}
    assert x.shape == (N_NODES, D) and edge_index.shape == (2, N_EDGES)

    prep = host_prep(x, edge_index, N_CORES)
    key = _build_key(prep)
    if key not in _CACHE:
        _CACHE[key] = build_program(*key, N_CORES)
    nc = _CACHE[key]
    in_maps = make_in_maps(prep, params)
    res = run_bass_kernel_spmd(
        nc, in_maps, core_ids=list(range(N_CORES)), trace=False
    )
    out = np.concatenate(
        [res.results[k]["out"] for k in range(N_CORES)], axis=0
    ).astype(np.float32)
    return out


# revision 7
# speedup vs baseline: 1.3640x; 1.3640x over previous
"""Self-contained GraphSAGE (3-layer, mean-aggr) Bass/Tile kernel for 8x TRN2.

kernel(**inputs) takes the FULL inputs (x [50000,128] f32, edge_index
[2,800000] i32, weights/biases) and returns the full [50000,64] f32 output.

Sharding: nodes split 8 ways; edges partitioned by destination shard; per
layer an AllGather of bf16 features; per-window bulk dma_gather of source
rows (int16 indices, split at row 32768 into lo/hi range gathers) and a
one-hot-matmul segment-mean on the tensor engine.
"""


import math

import numpy as np
from ml_dtypes import bfloat16

import concourse.bacc as bacc
import concourse.bass as bass
import concourse.tile as tile
from concourse import library_config, mybir
from concourse.bass_utils import run_bass_kernel_spmd


def _ensure_ntff_hook():
    """The agent image's ``antenv`` lacks ``axon_hooks``; synthesize it and
    install the ctypes-based NTFF profile hook so trace=True works."""
    try:
        from antenv.axon_hooks import get_axon_ntff_profile_hook  # noqa: F401
        return
    except ImportError:
        pass
    import sys
    import types

    mod = types.ModuleType("antenv.axon_hooks")
    _hook = [None]
    mod.set_axon_ntff_profile_hook = lambda h: _hook.__setitem__(0, h)
    mod.get_axon_ntff_profile_hook = lambda: _hook[0]
    sys.modules["antenv.axon_hooks"] = mod
    try:
        import antenv

        antenv.axon_hooks = mod
    except ImportError:
        pass
    try:
        from trn_agent_boot.trn_boot import _ntff_profile_via_ctypes

        so_path = "/opt/axon/libaxon_pjrt.so"
        hook = _ntff_profile_via_ctypes(so_path)
        if hook is not None:
            mod.set_axon_ntff_profile_hook(hook)
    except Exception:
        pass


_ensure_ntff_hook()

F32 = mybir.dt.float32
BF16 = mybir.dt.bfloat16
I16 = mybir.dt.int16
AF = mybir.ActivationFunctionType
OP = mybir.AluOpType

WN = 128   # window (dst-node tile) size
D = 128    # feature dim (layers 0/1 output, all layer inputs)
DOUT = 64
SPLIT = 32768  # int16 index range split
GB = 5     # windows per gather batch


def host_prep(x, edge_index, n_cores):
    """Build per-core host-side arrays. Edges are bucketed per dst window,
    split into lo/hi src ranges (int16 index limit), sorted by src for HBM
    locality, and laid out in gather-batch order."""
    N, d = x.shape
    assert d == D
    NS = N // n_cores
    W = math.ceil(NS / WN)
    src = edge_index[0].astype(np.int64)
    dst = edge_index[1].astype(np.int64)

    deg = np.bincount(dst, minlength=N).astype(np.float32)
    inv = (1.0 / np.maximum(deg, 1.0)).astype(np.float32)

    order = np.argsort(dst, kind="stable")
    srcs = src[order]
    dsts = dst[order]
    bounds = np.searchsorted(dsts, np.arange(n_cores + 1) * NS)

    # per (core, window, lo/hi) edge lists
    per_core = []
    nlo = np.zeros((n_cores, W), dtype=np.int64)
    nhi = np.zeros((n_cores, W), dtype=np.int64)
    for k in range(n_cores):
        lo_, hi_ = bounds[k], bounds[k + 1]
        es = srcs[lo_:hi_]
        ed = dsts[lo_:hi_] - k * NS
        win = ed // WN
        wstart = np.searchsorted(win, np.arange(W + 1))
        wins = []
        for w in range(W):
            a, b = wstart[w], wstart[w + 1]
            ws, wd = es[a:b], ed[a:b] % WN
            is_lo = ws < SPLIT
            lo_s, lo_d = ws[is_lo], wd[is_lo]
            hi_s, hi_d = ws[~is_lo], wd[~is_lo]
            olo = np.argsort(lo_s, kind="stable")
            ohi = np.argsort(hi_s, kind="stable")
            wins.append((lo_s[olo], lo_d[olo], hi_s[ohi], hi_d[ohi]))
            nlo[k, w] = lo_s.shape[0]
            nhi[k, w] = hi_s.shape[0]
        per_core.append(wins)

    cpwl = np.maximum(1, np.ceil(nlo.max(axis=0) / 128).astype(np.int64))
    cpwh = np.ceil(nhi.max(axis=0) / 128).astype(np.int64)

    # global chunk-column layout, per batch: [lo chunks per window][hi chunks]
    batches = []  # (w0, w1, lo_start, lo_n, hi_start, hi_n)
    lo_col = np.zeros(W, dtype=np.int64)
    hi_col = np.zeros(W, dtype=np.int64)
    cur = 0
    for b0 in range(0, W, GB):
        b1 = min(b0 + GB, W)
        lo_start = cur
        for w in range(b0, b1):
            lo_col[w] = cur
            cur += cpwl[w]
        hi_start = cur
        for w in range(b0, b1):
            hi_col[w] = cur
            cur += cpwh[w]
        batches.append(
            (b0, b1, int(lo_start), int(hi_start - lo_start), int(hi_start),
             int(cur - hi_start))
        )
    T = int(cur)

    idx_l, dstl_l, invd_l, xt_l = [], [], [], []
    for k in range(n_cores):
        idx_flat = np.zeros(T * 128, dtype=np.int16)
        dst_flat = np.full(T * 128, 240.0, dtype=np.float32)
        for w in range(W):
            lo_s, lo_d, hi_s, hi_d = per_core[k][w]
            s0 = lo_col[w] * 128
            idx_flat[s0 : s0 + len(lo_s)] = lo_s.astype(np.int16)
            dst_flat[s0 : s0 + len(lo_d)] = lo_d
            s0 = hi_col[w] * 128
            idx_flat[s0 : s0 + len(hi_s)] = (hi_s - SPLIT).astype(np.int16)
            dst_flat[s0 : s0 + len(hi_d)] = hi_d
        idx_l.append(
            np.ascontiguousarray(np.tile(idx_flat.reshape(T * 8, 16).T, (8, 1)))
        )
        dstl_l.append(
            np.ascontiguousarray(dst_flat.reshape(T, 128).T.astype(bfloat16))
        )

        v = np.zeros(W * WN, dtype=np.float32)
        v[:NS] = inv[k * NS : (k + 1) * NS]
        invd_l.append(np.ascontiguousarray(np.broadcast_to(v, (128, W * WN))))

        xt = np.zeros((128, W * WN), dtype=np.float32)
        xt[:, :NS] = x[k * NS : (k + 1) * NS].T
        xt_l.append(xt)

    iota = np.ascontiguousarray(
        np.broadcast_to(np.arange(WN, dtype=np.float32), (128, WN)).astype(bfloat16)
    )
    return dict(
        N=N, NS=NS, W=W, T=T, n_cores=n_cores,
        CPWL=tuple(int(c) for c in cpwl), CPWH=tuple(int(c) for c in cpwh),
        BATCHES=tuple(batches),
        LO_COL=tuple(int(c) for c in lo_col), HI_COL=tuple(int(c) for c in hi_col),
        idx=idx_l, dstl=dstl_l, invd=invd_l, xt=xt_l,
        x_bf16=np.ascontiguousarray(x.astype(bfloat16)).view(np.float32),
        iota=iota,
    )


def build_program(N, NS, W, T, CPWL, CPWH, BATCHES, LO_COL, HI_COL, n_cores,
                  mm_bufs=2, g_bufs=3, shared_ag=True):
    """Build the Bass/Tile SPMD program."""
    nc = bacc.Bacc(
        "TRN2", target_bir_lowering=False, debug=False, num_devices=n_cores,
        num_swdge_queues=4,
    )

    # ---- I/O ----
    xg = nc.dram_tensor("xg", [N, D // 2], F32, kind="ExternalInput")
    xt_in = nc.dram_tensor("xt", [128, W * WN], F32, kind="ExternalInput")
    idx_in = nc.dram_tensor("idx", [128, T * 8], I16, kind="ExternalInput")
    dstl_in = nc.dram_tensor("dstl", [128, T], BF16, kind="ExternalInput")
    invd_in = nc.dram_tensor("invd", [128, W * WN], F32, kind="ExternalInput")
    iota_in = nc.dram_tensor("iota", [128, WN], BF16, kind="ExternalInput")
    w_in = {}
    for i, do in ((0, D), (1, D), (2, DOUT)):
        w_in[f"wl{i}"] = nc.dram_tensor(f"wl{i}", [D, do], F32, kind="ExternalInput")
        w_in[f"wr{i}"] = nc.dram_tensor(f"wr{i}", [D, do], F32, kind="ExternalInput")
    bl0_in = nc.dram_tensor("bl0", [128, 1], F32, kind="ExternalInput")
    bl1_in = nc.dram_tensor("bl1", [128, 1], F32, kind="ExternalInput")
    b2b_in = nc.dram_tensor("b2b", [128, DOUT], F32, kind="ExternalInput")
    ident_in = nc.dram_tensor("ident", [128, 128], F32, kind="ExternalInput")
    out = nc.dram_tensor("out", [NS, DOUT], F32, kind="ExternalOutput")

    groups = [list(range(n_cores))]

    with tile.TileContext(nc) as tc:
        with (
            tc.tile_pool(name="const", bufs=1) as cpool,
            tc.tile_pool(name="state", bufs=1) as spool,
            tc.tile_pool(name="gather", bufs=g_bufs) as gpool,
            tc.tile_pool(name="pbuild", bufs=4) as ppool,
            tc.tile_pool(name="small", bufs=mm_bufs * 2) as smpool,
            tc.tile_pool(name="psA", bufs=mm_bufs, space="PSUM") as psA,
            tc.tile_pool(name="psY", bufs=mm_bufs, space="PSUM") as psY,
            tc.tile_pool(name="psR", bufs=mm_bufs, space="PSUM") as psR,
            tc.tile_pool(name="dram", bufs=1, space="DRAM") as dpool,
        ):
            nc.gpsimd.load_library(library_config.mlp)

            # ---- constants / resident state ----
            iota_sb = cpool.tile([128, WN], BF16)
            nc.sync.dma_start(out=iota_sb[:], in_=iota_in[:, :])
            ident_sb = cpool.tile([128, 128], F32)
            nc.sync.dma_start(out=ident_sb[:], in_=ident_in[:, :])
            idx_sb = cpool.tile([128, T * 8], I16)
            nc.sync.dma_start(out=idx_sb[:], in_=idx_in[:, :])
            dstl_sb = cpool.tile([128, T], BF16)
            nc.sync.dma_start(out=dstl_sb[:], in_=dstl_in[:, :])
            invd_sb = cpool.tile([128, W * WN], F32)
            nc.sync.dma_start(out=invd_sb[:], in_=invd_in[:, :])
            w_sb = {}
            for name, t in w_in.items():
                w_sb[name] = cpool.tile(list(t.shape), F32, name=f"{name}_sb")
                nc.sync.dma_start(out=w_sb[name][:], in_=t[:, :])
            bl_sb = [cpool.tile([128, 1], F32, name=f"blc{i}_sb") for i in range(2)]
            nc.sync.dma_start(out=bl_sb[0][:], in_=bl0_in[:, :])
            nc.sync.dma_start(out=bl_sb[1][:], in_=bl1_in[:, :])
            b2b_sb = cpool.tile([128, DOUT], F32)
            nc.sync.dma_start(out=b2b_sb[:], in_=b2b_in[:, :])

            ht = [
                spool.tile([128, W * WN], F32, name="ht0"),
                spool.tile([128, W * WN], F32, name="ht1"),
            ]
            nc.sync.dma_start(out=ht[0][:], in_=xt_in[:, :])

            ag_in = dpool.tile([NS, D // 2], F32, name="ag_in")
            ag_space = "Shared" if (n_cores > 4 and shared_ag) else "Local"
            h_full = [
                dpool.tile([N, D // 2], F32, name="h1", addr_space=ag_space),
                dpool.tile([N, D // 2], F32, name="h2", addr_space=ag_space),
            ]

            last_rows = NS - (W - 1) * WN
            self_q = [0]  # SWDGE queue rotation counter

            for L in range(3):
                table = xg if L == 0 else h_full[L - 1]
                cur = ht[L % 2]
                nxt = ht[(L + 1) % 2]
                wl = w_sb[f"wl{L}"]
                wr = w_sb[f"wr{L}"]
                tab_lo = table[:SPLIT, :].bitcast(BF16)
                tab_hi = table[SPLIT:, :].bitcast(BF16)
                for (b0, b1, lo_start, lo_n, hi_start, hi_n) in BATCHES:
                    Tb = lo_n + hi_n
                    mw = gpool.tile([128, Tb, 128], BF16, name="mw", tag="mw")

                    # slice each lo/hi run into <=8-chunk (1024-idx) calls —
                    # the ucode packet limit — rotated over the 4 SWDGE queues
                    def gcalls(gstart, nch, col0, tab_ap, mw=mw):
                        c = 0
                        while c < nch:
                            pc = min(8, nch - c)
                            g0 = gstart + c
                            nc.gpsimd.dma_gather(
                                mw[:, col0 + c : col0 + c + pc, :], tab_ap,
                                idx_sb[:, g0 * 8 : (g0 + pc) * 8],
                                pc * 128, pc * 128, 128,
                                queue_num=self_q[0] % 4,
                            )
                            self_q[0] += 1
                            c += pc

                    gcalls(lo_start, lo_n, 0, tab_lo)
                    gcalls(hi_start, hi_n, lo_n, tab_hi)
                    for w in range(b0, b1):
                        rows = WN if w < W - 1 else last_rows
                        cl, ch = CPWL[w], CPWH[w]
                        NCH = cl + ch
                        # one-hot P for this window's chunks (lo then hi)
                        pw = ppool.tile([128, NCH * WN], BF16, name="pw", tag="pw")
                        nc.vector.tensor_tensor(
                            out=pw[:, : cl * WN].rearrange(
                                "p (c n) -> p c n", n=WN
                            ),
                            in0=dstl_sb[
                                :, LO_COL[w] : LO_COL[w] + cl, None
                            ].to_broadcast([128, cl, WN]),
                            in1=iota_sb[:, None, :].to_broadcast([128, cl, WN]),
                            op=OP.is_equal,
                        )
                        if ch:
                            nc.vector.tensor_tensor(
                                out=pw[:, cl * WN :].rearrange(
                                    "p (c n) -> p c n", n=WN
                                ),
                                in0=dstl_sb[
                                    :, HI_COL[w] : HI_COL[w] + ch, None
                                ].to_broadcast([128, ch, WN]),
                                in1=iota_sb[:, None, :].to_broadcast([128, ch, WN]),
                                op=OP.is_equal,
                            )
                        # segment-sum: PSUM_A[feat, node] += M_c.T @ P_c
                        pa = psA.tile([128, WN], F32, name="pa")
                        mw_lo0 = LO_COL[w] - lo_start
                        mw_hi0 = lo_n + (HI_COL[w] - hi_start)
                        for c in range(NCH):
                            mcol = (mw_lo0 + c) if c < cl else (mw_hi0 + c - cl)
                            nc.tensor.matmul(
                                out=pa[:],
                                lhsT=mw[:, mcol, :],
                                rhs=pw[:, c * WN : (c + 1) * WN],
                                start=(c == 0),
                                stop=(c == NCH - 1),
                            )
                        # normalize (segment mean) while copying PSUM->SBUF
                        aggt = smpool.tile([128, WN], F32, name="aggt")
                        nc.vector.tensor_tensor(
                            out=aggt[:],
                            in0=pa[:],
                            in1=invd_sb[:, w * WN : (w + 1) * WN],
                            op=OP.mult,
                        )
                        if L < 2:
                            # yT = Wl.T @ aggT + Wr.T @ hT_win
                            py = psY.tile([128, WN], F32, name="py")
                            nc.tensor.matmul(
                                out=py[:], lhsT=wl[:], rhs=aggt[:],
                                start=True, stop=False,
                            )
                            nc.tensor.matmul(
                                out=py[:],
                                lhsT=wr[:],
                                rhs=cur[:, w * WN : (w + 1) * WN],
                                start=False,
                                stop=True,
                            )
                            # hT_next = relu(yT + b) (bias per-partition)
                            nc.scalar.activation(
                                out=nxt[:, w * WN : (w + 1) * WN],
                                in_=py[:],
                                func=AF.Relu,
                                bias=bl_sb[L][:, :1],
                            )
                            # row-major bf16 copy for the allgather input
                            pr = psR.tile([128, WN], F32, name="pr")
                            nc.tensor.transpose(
                                out=pr[:],
                                in_=nxt[:, w * WN : (w + 1) * WN],
                                identity=ident_sb[:],
                            )
                            hrow = smpool.tile([128, D], BF16, name="hrow")
                            nc.vector.tensor_copy(out=hrow[:], in_=pr[:])
                            nc.sync.dma_start(
                                out=ag_in[w * WN : w * WN + rows, :],
                                in_=hrow[:rows, :].bitcast(F32),
                            )
                        else:
                            # final layer: out = aggT.T@Wl2 + hT.T@Wr2 + b2
                            pf = psY.tile([128, DOUT], F32, name="pf")
                            nc.tensor.matmul(
                                out=pf[:], lhsT=aggt[:], rhs=w_sb["wl2"][:],
                                start=True, stop=False,
                            )
                            nc.tensor.matmul(
                                out=pf[:],
                                lhsT=cur[:, w * WN : (w + 1) * WN],
                                rhs=w_sb["wr2"][:],
                                start=False,
                                stop=True,
                            )
                            osb = smpool.tile([128, DOUT], F32, name="osb")
                            nc.vector.tensor_tensor(
                                out=osb[:], in0=pf[:], in1=b2b_sb[:], op=OP.add
                            )
                            nc.sync.dma_start(
                                out=out[w * WN : w * WN + rows, :],
                                in_=osb[:rows, :],
                            )
                if L < 2:
                    nc.gpsimd.collective_compute(
                        "AllGather",
                        OP.bypass,
                        replica_groups=groups,
                        ins=[ag_in[:, :]],
                        outs=[h_full[L][:, :]],
                    )

    nc.compile()
    return nc


def make_in_maps(prep, params):
    """params: dict with Wl0,bl0,Wr0,...  Returns list of per-core in_maps."""
    n_cores = prep["n_cores"]
    ident = np.eye(128, dtype=np.float32)
    common = dict(
        xg=prep["x_bf16"],
        iota=prep["iota"],
        ident=ident,
        bl0=np.asarray(params["bl0"], np.float32).reshape(128, 1),
        bl1=np.asarray(params["bl1"], np.float32).reshape(128, 1),
        b2b=np.ascontiguousarray(
            np.broadcast_to(np.asarray(params["bl2"], np.float32), (128, DOUT))
        ),
    )
    for i in range(3):
        common[f"wl{i}"] = np.asarray(params[f"Wl{i}"], np.float32)
        common[f"wr{i}"] = np.asarray(params[f"Wr{i}"], np.float32)
    return [
        dict(
            common,
            xt=prep["xt"][k],
            idx=prep["idx"][k],
            dstl=prep["dstl"][k],
            invd=prep["invd"][k],
        )
        for k in range(n_cores)
    ]


def _build_key(prep):
    return (prep["N"], prep["NS"], prep["W"], prep["T"], prep["CPWL"],
            prep["CPWH"], prep["BATCHES"], prep["LO_COL"], prep["HI_COL"])


def run(x, edge_index, params, n_cores=8, trace=False, prep=None, nc=None):
    if prep is None:
        prep = host_prep(np.asarray(x, np.float32), np.asarray(edge_index), n_cores)
    if nc is None:
        nc = build_program(*_build_key(prep), n_cores)
    in_maps = make_in_maps(prep, params)
    res = run_bass_kernel_spmd(
        nc, in_maps, core_ids=list(range(n_cores)), trace=trace
    )
    outs = [res.results[k]["out"] for k in range(n_cores)]
    return np.concatenate(outs, axis=0), res


_CACHE = {}

N_NODES = 50000
N_EDGES = 800000
N_CORES = 8


def kernel(**inputs):
    x = np.asarray(inputs["x"], dtype=np.float32)
    edge_index = np.asarray(inputs["edge_index"])
    params = {k: np.asarray(v) for k, v in inputs.items()
              if k not in ("x", "edge_index")}
    assert x.shape == (N_NODES, D) and edge_index.shape == (2, N_EDGES)

    prep = host_prep(x, edge_index, N_CORES)
    key = _build_key(prep)
    if key not in _CACHE:
        _CACHE[key] = build_program(*key, N_CORES)
    nc = _CACHE[key]
    in_maps = make_in_maps(prep, params)
    res = run_bass_kernel_spmd(
        nc, in_maps, core_ids=list(range(N_CORES)), trace=False
    )
    out = np.concatenate(
        [res.results[k]["out"] for k in range(N_CORES)], axis=0
    ).astype(np.float32)
    return out


# revision 13
# speedup vs baseline: 1.3830x; 1.0139x over previous
"""Self-contained GraphSAGE (3-layer, mean-aggr) Bass/Tile kernel for 8x TRN2.

kernel(**inputs) takes the FULL inputs (x [50000,128] f32, edge_index
[2,800000] i32, weights/biases) and returns the full [50000,64] f32 output.

Sharding: nodes split 8 ways; edges partitioned by destination shard; per
layer an AllGather of bf16 features; per-window bulk dma_gather of source
rows (int16 indices, split at row 32768 into lo/hi range gathers) and a
one-hot-matmul segment-mean on the tensor engine.
"""


import math

import numpy as np
from ml_dtypes import bfloat16

import concourse.bacc as bacc
import concourse.bass as bass
import concourse.tile as tile
from concourse import library_config, mybir
from concourse.bass_utils import run_bass_kernel_spmd


def _ensure_ntff_hook():
    """The agent image's ``antenv`` lacks ``axon_hooks``; synthesize it and
    install the ctypes-based NTFF profile hook so trace=True works."""
    try:
        from antenv.axon_hooks import get_axon_ntff_profile_hook  # noqa: F401
        return
    except ImportError:
        pass
    import sys
    import types

    mod = types.ModuleType("antenv.axon_hooks")
    _hook = [None]
    mod.set_axon_ntff_profile_hook = lambda h: _hook.__setitem__(0, h)
    mod.get_axon_ntff_profile_hook = lambda: _hook[0]
    sys.modules["antenv.axon_hooks"] = mod
    try:
        import antenv

        antenv.axon_hooks = mod
    except ImportError:
        pass
    try:
        from trn_agent_boot.trn_boot import _ntff_profile_via_ctypes

        so_path = "/opt/axon/libaxon_pjrt.so"
        hook = _ntff_profile_via_ctypes(so_path)
        if hook is not None:
            mod.set_axon_ntff_profile_hook(hook)
    except Exception:
        pass


_ensure_ntff_hook()

F32 = mybir.dt.float32
BF16 = mybir.dt.bfloat16
I16 = mybir.dt.int16
AF = mybir.ActivationFunctionType
OP = mybir.AluOpType

WN = 128   # window (dst-node tile) size
D = 128    # feature dim (layers 0/1 output, all layer inputs)
DOUT = 64
SPLIT = 25600  # lo/hi gather split = AG half boundary (int16-safe)
GB = 5     # windows per gather batch


def host_prep(x, edge_index, n_cores):
    """Build per-core host-side arrays. Edges are bucketed per dst window,
    split into lo/hi src ranges (int16 index limit), sorted by src for HBM
    locality, and laid out in gather-batch order."""
    N, d = x.shape
    assert d == D
    NS = N // n_cores
    W = math.ceil(NS / WN)
    src = edge_index[0].astype(np.int64)
    dst = edge_index[1].astype(np.int64)

    deg = np.bincount(dst, minlength=N).astype(np.float32)
    inv = (1.0 / np.maximum(deg, 1.0)).astype(np.float32)

    # table rows live in split-allgather order: first-half windows of every
    # core first ([8*H1 rows]), then second halves — lets AG fire in halves
    H1 = (W // 2 + 1) * WN  # 3200 (windows 0..24)
    H2 = NS - H1
    N1 = n_cores * H1
    k_of = src // NS
    r_of = src % NS
    src_rm = np.where(r_of < H1, k_of * H1 + r_of, N1 + k_of * H2 + (r_of - H1))

    order = np.argsort(dst, kind="stable")
    srcs = src_rm[order]
    dsts = dst[order]
    bounds = np.searchsorted(dsts, np.arange(n_cores + 1) * NS)

    # per (core, window, lo/hi) edge lists
    per_core = []
    nlo = np.zeros((n_cores, W), dtype=np.int64)
    nhi = np.zeros((n_cores, W), dtype=np.int64)
    for k in range(n_cores):
        lo_, hi_ = bounds[k], bounds[k + 1]
        es = srcs[lo_:hi_]
        ed = dsts[lo_:hi_] - k * NS
        win = ed // WN
        wstart = np.searchsorted(win, np.arange(W + 1))
        wins = []
        for w in range(W):
            a, b = wstart[w], wstart[w + 1]
            ws, wd = es[a:b], ed[a:b] % WN
            is_lo = ws < SPLIT
            lo_s, lo_d = ws[is_lo], wd[is_lo]
            hi_s, hi_d = ws[~is_lo], wd[~is_lo]
            olo = np.argsort(lo_s, kind="stable")
            ohi = np.argsort(hi_s, kind="stable")
            wins.append((lo_s[olo], lo_d[olo], hi_s[ohi], hi_d[ohi]))
            nlo[k, w] = lo_s.shape[0]
            nhi[k, w] = hi_s.shape[0]
        per_core.append(wins)

    cpwl = np.maximum(1, np.ceil(nlo.max(axis=0) / 128).astype(np.int64))
    cpwh = np.ceil(nhi.max(axis=0) / 128).astype(np.int64)

    # global chunk-column layout, per batch: [lo chunks per window][hi chunks]
    batches = []  # (w0, w1, lo_start, lo_n, hi_start, hi_n)
    lo_col = np.zeros(W, dtype=np.int64)
    hi_col = np.zeros(W, dtype=np.int64)
    cur = 0
    for b0 in range(0, W, GB):
        b1 = min(b0 + GB, W)
        lo_start = cur
        for w in range(b0, b1):
            lo_col[w] = cur
            cur += cpwl[w]
        hi_start = cur
        for w in range(b0, b1):
            hi_col[w] = cur
            cur += cpwh[w]
        batches.append(
            (b0, b1, int(lo_start), int(hi_start - lo_start), int(hi_start),
             int(cur - hi_start))
        )
    T = int(cur)

    idx_l, dstl_l, invd_l, xt_l = [], [], [], []
    for k in range(n_cores):
        idx_flat = np.zeros(T * 128, dtype=np.int16)
        dst_flat = np.full(T * 128, 240.0, dtype=np.float32)
        for w in range(W):
            lo_s, lo_d, hi_s, hi_d = per_core[k][w]
            s0 = lo_col[w] * 128
            idx_flat[s0 : s0 + len(lo_s)] = lo_s.astype(np.int16)
            dst_flat[s0 : s0 + len(lo_d)] = lo_d
            s0 = hi_col[w] * 128
            idx_flat[s0 : s0 + len(hi_s)] = (hi_s - SPLIT).astype(np.int16)
            dst_flat[s0 : s0 + len(hi_d)] = hi_d
        idx_l.append(
            np.ascontiguousarray(np.tile(idx_flat.reshape(T * 8, 16).T, (8, 1)))
        )
        dstl_l.append(
            np.ascontiguousarray(dst_flat.reshape(T, 128).T.astype(bfloat16))
        )

        v = np.zeros(W * WN, dtype=np.float32)
        v[:NS] = inv[k * NS : (k + 1) * NS]
        invd_l.append(np.ascontiguousarray(np.broadcast_to(v, (128, W * WN))))

        xt = np.zeros((128, W * WN), dtype=np.float32)
        xt[:, :NS] = x[k * NS : (k + 1) * NS].T
        xt_l.append(xt)

    iota = np.ascontiguousarray(
        np.broadcast_to(np.arange(WN, dtype=np.float32), (128, WN)).astype(bfloat16)
    )
    nodes = np.arange(N)
    kk = nodes // NS
    rr = nodes % NS
    new_of = np.where(rr < H1, kk * H1 + rr, N1 + kk * H2 + (rr - H1))
    xg_r = np.empty((N, D), dtype=bfloat16)
    xg_r[new_of] = x.astype(bfloat16)
    return dict(
        N=N, NS=NS, W=W, T=T, n_cores=n_cores, H1=H1,
        CPWL=tuple(int(c) for c in cpwl), CPWH=tuple(int(c) for c in cpwh),
        BATCHES=tuple(batches),
        LO_COL=tuple(int(c) for c in lo_col), HI_COL=tuple(int(c) for c in hi_col),
        idx=idx_l, dstl=dstl_l, invd=invd_l, xt=xt_l,
        x_bf16=np.ascontiguousarray(xg_r).view(np.float32),
        iota=iota,
    )


def build_program(N, NS, W, T, CPWL, CPWH, BATCHES, LO_COL, HI_COL, H1,
                  n_cores, mm_bufs=2, g_bufs=3, shared_ag=True):
    """Build the Bass/Tile SPMD program."""
    nc = bacc.Bacc(
        "TRN2", target_bir_lowering=False, debug=False, num_devices=n_cores,
        num_swdge_queues=4,
    )

    # ---- I/O ----
    xg = nc.dram_tensor("xg", [N, D // 2], F32, kind="ExternalInput")
    xt_in = nc.dram_tensor("xt", [128, W * WN], F32, kind="ExternalInput")
    idx_in = nc.dram_tensor("idx", [128, T * 8], I16, kind="ExternalInput")
    dstl_in = nc.dram_tensor("dstl", [128, T], BF16, kind="ExternalInput")
    invd_in = nc.dram_tensor("invd", [128, W * WN], F32, kind="ExternalInput")
    iota_in = nc.dram_tensor("iota", [128, WN], BF16, kind="ExternalInput")
    w_in = {}
    for i, do in ((0, D), (1, D), (2, DOUT)):
        w_in[f"wl{i}"] = nc.dram_tensor(f"wl{i}", [D, do], F32, kind="ExternalInput")
        w_in[f"wr{i}"] = nc.dram_tensor(f"wr{i}", [D, do], F32, kind="ExternalInput")
    bl0_in = nc.dram_tensor("bl0", [128, 1], F32, kind="ExternalInput")
    bl1_in = nc.dram_tensor("bl1", [128, 1], F32, kind="ExternalInput")
    b2b_in = nc.dram_tensor("b2b", [128, DOUT], F32, kind="ExternalInput")
    ident_in = nc.dram_tensor("ident", [128, 128], F32, kind="ExternalInput")
    out = nc.dram_tensor("out", [NS, DOUT], F32, kind="ExternalOutput")

    groups = [list(range(n_cores))]

    with tile.TileContext(nc) as tc:
        with (
            tc.tile_pool(name="const", bufs=1) as cpool,
            tc.tile_pool(name="state", bufs=1) as spool,
            tc.tile_pool(name="gather", bufs=g_bufs) as gpool,
            tc.tile_pool(name="pbuild", bufs=4) as ppool,
            tc.tile_pool(name="small", bufs=mm_bufs * 2) as smpool,
            tc.tile_pool(name="psA", bufs=mm_bufs, space="PSUM") as psA,
            tc.tile_pool(name="psY", bufs=mm_bufs, space="PSUM") as psY,
            tc.tile_pool(name="psR", bufs=mm_bufs, space="PSUM") as psR,
            tc.tile_pool(name="dram", bufs=1, space="DRAM") as dpool,
        ):
            nc.gpsimd.load_library(library_config.mlp)

            # ---- constants / resident state ----
            iota_sb = cpool.tile([128, WN], BF16)
            nc.sync.dma_start(out=iota_sb[:], in_=iota_in[:, :])
            ident_sb = cpool.tile([128, 128], F32)
            nc.sync.dma_start(out=ident_sb[:], in_=ident_in[:, :])
            idx_sb = cpool.tile([128, T * 8], I16)
            nc.sync.dma_start(out=idx_sb[:], in_=idx_in[:, :])
            dstl_sb = cpool.tile([128, T], BF16)
            nc.sync.dma_start(out=dstl_sb[:], in_=dstl_in[:, :])
            invd_sb = cpool.tile([128, W * WN], F32)
            nc.sync.dma_start(out=invd_sb[:], in_=invd_in[:, :])
            w_sb = {}
            for name, t in w_in.items():
                w_sb[name] = cpool.tile(list(t.shape), F32, name=f"{name}_sb")
                nc.sync.dma_start(out=w_sb[name][:], in_=t[:, :])
            bl_sb = [cpool.tile([128, 1], F32, name=f"blc{i}_sb") for i in range(2)]
            nc.sync.dma_start(out=bl_sb[0][:], in_=bl0_in[:, :])
            nc.sync.dma_start(out=bl_sb[1][:], in_=bl1_in[:, :])
            b2b_sb = cpool.tile([128, DOUT], F32)
            nc.sync.dma_start(out=b2b_sb[:], in_=b2b_in[:, :])

            ht = [
                spool.tile([128, W * WN], F32, name="ht0"),
                spool.tile([128, W * WN], F32, name="ht1"),
            ]
            nc.sync.dma_start(out=ht[0][:], in_=xt_in[:, :])

            ag_in = dpool.tile([NS, D // 2], F32, name="ag_in")
            ag_space = "Shared" if (n_cores > 4 and shared_ag) else "Local"
            N1 = n_cores * H1
            h_a = [
                dpool.tile([N1, D // 2], F32, name="h1a", addr_space=ag_space),
                dpool.tile([N1, D // 2], F32, name="h2a", addr_space=ag_space),
            ]
            h_b = [
                dpool.tile([N - N1, D // 2], F32, name="h1b", addr_space=ag_space),
                dpool.tile([N - N1, D // 2], F32, name="h2b", addr_space=ag_space),
            ]

            last_rows = NS - (W - 1) * WN
            self_q = [0]  # SWDGE queue rotation counter

            for L in range(3):
                cur = ht[L % 2]
                nxt = ht[(L + 1) % 2]
                wl = w_sb[f"wl{L}"]
                wr = w_sb[f"wr{L}"]
                if L == 0:
                    tab_lo = xg[:SPLIT, :].bitcast(BF16)
                    tab_hi = xg[SPLIT:, :].bitcast(BF16)
                else:
                    tab_lo = h_a[L - 1][:, :].bitcast(BF16)
                    tab_hi = h_b[L - 1][:, :].bitcast(BF16)
                for (b0, b1, lo_start, lo_n, hi_start, hi_n) in BATCHES:
                    Tb = lo_n + hi_n
                    mw = gpool.tile([128, Tb, 128], BF16, name="mw", tag="mw")

                    # slice each lo/hi run into <=8-chunk (1024-idx) calls —
                    # the ucode packet limit — rotated over the 4 SWDGE queues
                    def gcalls(gstart, nch, col0, tab_ap, mw=mw):
                        c = 0
                        while c < nch:
                            pc = min(8, nch - c)
                            g0 = gstart + c
                            nc.gpsimd.dma_gather(
                                mw[:, col0 + c : col0 + c + pc, :], tab_ap,
                                idx_sb[:, g0 * 8 : (g0 + pc) * 8],
                                pc * 128, pc * 128, 128,
                                queue_num=self_q[0] % 4,
                            )
                            self_q[0] += 1
                            c += pc

                    gcalls(lo_start, lo_n, 0, tab_lo)
                    gcalls(hi_start, hi_n, lo_n, tab_hi)
                    if L < 2 and b0 == 30:
                        # first-half rows of every core are done: fire AG#1
                        # so it overlaps the remaining windows' compute
                        nc.gpsimd.collective_compute(
                            "AllGather",
                            OP.bypass,
                            replica_groups=groups,
                            ins=[ag_in[:H1, :]],
                            outs=[h_a[L][:, :]],
                        )
                    for w in range(b0, b1):
                        rows = WN if w < W - 1 else last_rows
                        cl, ch = CPWL[w], CPWH[w]
                        NCH = cl + ch
                        # one-hot P for this window's chunks (lo then hi)
                        pw = ppool.tile([128, NCH * WN], BF16, name="pw", tag="pw")
                        nc.vector.tensor_tensor(
                            out=pw[:, : cl * WN].rearrange(
                                "p (c n) -> p c n", n=WN
                            ),
                            in0=dstl_sb[
                                :, LO_COL[w] : LO_COL[w] + cl, None
                            ].to_broadcast([128, cl, WN]),
                            in1=iota_sb[:, None, :].to_broadcast([128, cl, WN]),
                            op=OP.is_equal,
                        )
                        if ch:
                            nc.vector.tensor_tensor(
                                out=pw[:, cl * WN :].rearrange(
                                    "p (c n) -> p c n", n=WN
                                ),
                                in0=dstl_sb[
                                    :, HI_COL[w] : HI_COL[w] + ch, None
                                ].to_broadcast([128, ch, WN]),
                                in1=iota_sb[:, None, :].to_broadcast([128, ch, WN]),
                                op=OP.is_equal,
                            )
                        # segment-sum: PSUM_A[feat, node] += M_c.T @ P_c
                        pa = psA.tile([128, WN], F32, name="pa")
                        mw_lo0 = LO_COL[w] - lo_start
                        mw_hi0 = lo_n + (HI_COL[w] - hi_start)
                        for c in range(NCH):
                            mcol = (mw_lo0 + c) if c < cl else (mw_hi0 + c - cl)
                            nc.tensor.matmul(
                                out=pa[:],
                                lhsT=mw[:, mcol, :],
                                rhs=pw[:, c * WN : (c + 1) * WN],
                                start=(c == 0),
                                stop=(c == NCH - 1),
                            )
                        # normalize (segment mean) while copying PSUM->SBUF
                        aggt = smpool.tile([128, WN], F32, name="aggt")
                        nc.vector.tensor_tensor(
                            out=aggt[:],
                            in0=pa[:],
                            in1=invd_sb[:, w * WN : (w + 1) * WN],
                            op=OP.mult,
                        )
                        if L < 2:
                            # yT = Wl.T @ aggT + Wr.T @ hT_win
                            py = psY.tile([128, WN], F32, name="py")
                            nc.tensor.matmul(
                                out=py[:], lhsT=wl[:], rhs=aggt[:],
                                start=True, stop=False,
                            )
                            nc.tensor.matmul(
                                out=py[:],
                                lhsT=wr[:],
                                rhs=cur[:, w * WN : (w + 1) * WN],
                                start=False,
                                stop=True,
                            )
                            # hT_next = relu(yT + b) (bias per-partition)
                            nc.scalar.activation(
                                out=nxt[:, w * WN : (w + 1) * WN],
                                in_=py[:],
                                func=AF.Relu,
                                bias=bl_sb[L][:, :1],
                            )
                            # row-major bf16 copy for the allgather input
                            pr = psR.tile([128, WN], F32, name="pr")
                            nc.tensor.transpose(
                                out=pr[:],
                                in_=nxt[:, w * WN : (w + 1) * WN],
                                identity=ident_sb[:],
                            )
                            hrow = smpool.tile([128, D], BF16, name="hrow")
                            nc.vector.tensor_copy(out=hrow[:], in_=pr[:])
                            nc.sync.dma_start(
                                out=ag_in[w * WN : w * WN + rows, :],
                                in_=hrow[:rows, :].bitcast(F32),
                            )
                        else:
                            # final layer: out = aggT.T@Wl2 + hT.T@Wr2 + b2
                            pf = psY.tile([128, DOUT], F32, name="pf")
                            nc.tensor.matmul(
                                out=pf[:], lhsT=aggt[:], rhs=w_sb["wl2"][:],
                                start=True, stop=False,
                            )
                            nc.tensor.matmul(
                                out=pf[:],
                                lhsT=cur[:, w * WN : (w + 1) * WN],
                                rhs=w_sb["wr2"][:],
                                start=False,
                                stop=True,
                            )
                            osb = smpool.tile([128, DOUT], F32, name="osb")
                            nc.vector.tensor_tensor(
                                out=osb[:], in0=pf[:], in1=b2b_sb[:], op=OP.add
                            )
                            nc.sync.dma_start(
                                out=out[w * WN : w * WN + rows, :],
                                in_=osb[:rows, :],
                            )
                if L < 2:
                    nc.gpsimd.collective_compute(
                        "AllGather",
                        OP.bypass,
                        replica_groups=groups,
                        ins=[ag_in[H1:, :]],
                        outs=[h_b[L][:, :]],
                    )

    nc.compile()
    return nc


def make_in_maps(prep, params):
    """params: dict with Wl0,bl0,Wr0,...  Returns list of per-core in_maps."""
    n_cores = prep["n_cores"]
    ident = np.eye(128, dtype=np.float32)
    common = dict(
        xg=prep["x_bf16"],
        iota=prep["iota"],
        ident=ident,
        bl0=np.asarray(params["bl0"], np.float32).reshape(128, 1),
        bl1=np.asarray(params["bl1"], np.float32).reshape(128, 1),
        b2b=np.ascontiguousarray(
            np.broadcast_to(np.asarray(params["bl2"], np.float32), (128, DOUT))
        ),
    )
    for i in range(3):
        common[f"wl{i}"] = np.asarray(params[f"Wl{i}"], np.float32)
        common[f"wr{i}"] = np.asarray(params[f"Wr{i}"], np.float32)
    return [
        dict(
            common,
            xt=prep["xt"][k],
            idx=prep["idx"][k],
            dstl=prep["dstl"][k],
            invd=prep["invd"][k],
        )
        for k in range(n_cores)
    ]


def _build_key(prep):
    return (prep["N"], prep["NS"], prep["W"], prep["T"], prep["CPWL"],
            prep["CPWH"], prep["BATCHES"], prep["LO_COL"], prep["HI_COL"],
            prep["H1"])


def run(x, edge_index, params, n_cores=8, trace=False, prep=None, nc=None):
    if prep is None:
        prep = host_prep(np.asarray(x, np.float32), np.asarray(edge_index), n_cores)
    if nc is None:
        nc = build_program(*_build_key(prep), n_cores)
    in_maps = make_in_maps(prep, params)
    res = run_bass_kernel_spmd(
        nc, in_maps, core_ids=list(range(n_cores)), trace=trace
    )
    outs = [res.results[k]["out"] for k in range(n_cores)]
    return np.concatenate(outs, axis=0), res


_CACHE = {}

N_NODES = 50000
N_EDGES = 800000
N_CORES = 8


def kernel(**inputs):
    x = np.asarray(inputs["x"], dtype=np.float32)
    edge_index = np.asarray(inputs["edge_index"])
    params = {k: np.asarray(v) for k, v in inputs.items()
              if k not in ("x", "edge_index")}
    assert x.shape == (N_NODES, D) and edge_index.shape == (2, N_EDGES)

    prep = host_prep(x, edge_index, N_CORES)
    key = _build_key(prep)
    if key not in _CACHE:
        _CACHE[key] = build_program(*key, N_CORES)
    nc = _CACHE[key]
    in_maps = make_in_maps(prep, params)
    res = run_bass_kernel_spmd(
        nc, in_maps, core_ids=list(range(N_CORES)), trace=False
    )
    out = np.concatenate(
        [res.results[k]["out"] for k in range(N_CORES)], axis=0
    ).astype(np.float32)
    return out
